# revision 24
# baseline (speedup 1.0000x reference)
"""Causal multi-head decoder attention for Trainium2, 8-core SPMD.

Sharding: tensor-parallel over heads. Core i owns output channels
[128*i, 128*i+128) of the QKV projections = heads {2i, 2i+1}, for both
batches. Each core computes q/k/v for its heads from the full x, runs
causal attention locally, and produces a partial out-projection
(y_local @ Wo_local.T) over its 128 y-channels. Host sums the 8
partials and adds bo.

Device layouts (per core):
  xT   [C, B*T]  (c on partitions)   - host pre-transposed
  wqT/wkT/wvT [C, 128]               - host pre-transposed W[ci].T
  woT  [128, C]                      - host Wo[:, ci].T
  q/k/v/y tiles [128, 1024] per (batch, T-half), channels on partitions
  scores kept transposed: P.T [k, q] so AV contracts k on partitions;
  softmax denominator comes free via a ones-column appended to V.

The two local heads' score matmuls contract over disjoint 64-partition
row strips (base partitions 0 and 64), so the PE runs them as
concurrent row-tiles; their scores land side by side in one PSUM tile
and share a single exp ACTIVATE per k-tile.

All matmul operands are float32r (full PE rate at moving dim >= 256,
~1e-4 matmul rel err vs fp32).
"""

import numpy as np
import ml_dtypes

import concourse.bass as bass
import concourse.mybir as mybir
import concourse.tile as tile
from concourse import bacc
from concourse.masks import make_identity
from concourse.bass_utils import run_bass_kernel_spmd

B, T, C, H, D = 2, 2048, 1024, 16, 64
P = 128
NTOK = B * T              # 4096
CT = C // P               # 8 contraction tiles for the projections
NCORES = 8
SCALE = 1.0 / float(np.sqrt(D))   # 1/8
TB = 512                  # phase-1 token block
NTB = NTOK // TB          # 8
KT = T // P               # 16 key tiles per (b, h)
QC = 512                  # attention q-chunk
NQC = T // QC             # 4
HF = 1024                 # half of T (activation tile width)
f32 = mybir.dt.float32
f32r = mybir.dt.float32r
bf16 = mybir.dt.bfloat16
EXP = mybir.ActivationFunctionType.Exp

_CACHED_NC = None


def _build_nc(repeat=1, phases=(1, 2, 3, 4)):
    nc = bacc.Bacc("TRN2", target_bir_lowering=False, debug=False)

    xT = nc.dram_tensor("xT", [C, NTOK], bf16, kind="ExternalInput")
    wqT = nc.dram_tensor("wqT", [C, P], bf16, kind="ExternalInput")
    wkT = nc.dram_tensor("wkT", [C, P], bf16, kind="ExternalInput")
    wvT = nc.dram_tensor("wvT", [C, P], bf16, kind="ExternalInput")
    woT = nc.dram_tensor("woT", [P, C], f32r, kind="ExternalInput")
    bq = nc.dram_tensor("bq", [P, 1], f32, kind="ExternalInput")
    bk = nc.dram_tensor("bk", [P, 1], f32, kind="ExternalInput")
    bv = nc.dram_tensor("bv", [P, 1], f32, kind="ExternalInput")
    out = nc.dram_tensor("out", [NTOK, C], f32, kind="ExternalOutput")

    # [j, u, p, n]: c-tile j, token-block u
    xT_t = xT.ap().rearrange("(j p) (u n) -> j u p n", p=P, n=TB)
    wqT_t = wqT.ap().rearrange("(j p) m -> j p m", p=P)
    wkT_t = wkT.ap().rearrange("(j p) m -> j p m", p=P)
    wvT_t = wvT.ap().rearrange("(j p) m -> j p m", p=P)
    out_t = out.ap().rearrange("(s p) c -> s p c", p=P)   # 32 row-tiles

    with tile.TileContext(nc) as tc:
        with (
            tc.tile_pool(name="cst", bufs=1) as cst,
            tc.tile_pool(name="xt", bufs=4) as xtp,
            tc.tile_pool(name="pt", bufs=6) as ptp,
            tc.tile_pool(name="sb", bufs=4) as sbp,
            tc.tile_pool(name="ps2", bufs=2, space="PSUM") as ps2,
            tc.tile_pool(name="psy", bufs=1, space="PSUM") as psy,
        ):
            # ---- DMA head: wq first, then the first x blocks, so the
            # first projection chain starts as early as possible ----
            w_tiles = {"q": []}
            for j in range(CT):
                t_ = cst.tile([P, P], bf16, name=f"wq{j}")
                nc.sync.dma_start(t_[:], wqT_t[j])
                w_tiles["q"].append(t_)
            prefetch = {}
            for u in (0, 1):
                xts = []
                for j in range(CT):
                    xt_ = xtp.tile([P, TB], bf16, tag=f"xt{j}",
                                   name=f"xt{j}_{u}_pre")
                    (nc.sync if j % 2 == 0 else nc.gpsimd).dma_start(
                        xt_[:], xT_t[j, u])
                    xts.append(xt_)
                prefetch[u] = xts
            for nm, src in (("k", wkT_t), ("v", wvT_t)):
                w_tiles[nm] = []
                for j in range(CT):
                    t_ = cst.tile([P, P], bf16, name=f"w{nm}{j}")
                    nc.sync.dma_start(t_[:], src[j])
                    w_tiles[nm].append(t_)
            bias = {}
            for nm, src in (("q", bq), ("k", bk), ("v", bv)):
                t_ = cst.tile([P, 1], f32, name=f"b{nm}")
                nc.sync.dma_start(t_[:], src.ap())
                bias[nm] = t_
            wo_sb = cst.tile([P, C], f32r, name="wo_sb")
            nc.sync.dma_start(wo_sb[:], woT.ap())

            # masks are built in f32 (gpsimd can't emit f32r), then
            # DVE-copied into f32r tiles (the copy rounds for the verifier)
            ident_f = cst.tile([P, P], f32, name="ident_f")
            make_identity(nc, ident_f[:])
            # lower-triangular keep mask (in P.T layout): keep q >= k
            tri_f = cst.tile([P, P], f32, name="tri_f")
            nc.gpsimd.memset(tri_f[:], 1.0)
            nc.gpsimd.affine_select(
                out=tri_f[:], in_=tri_f[:],
                compare_op=mybir.AluOpType.is_ge,
                fill=0.0, base=0,
                pattern=[[1, P]], channel_multiplier=-1,
            )
            trimask = cst.tile([P, P], bf16, name="trimask")
            nc.vector.tensor_copy(trimask[:], tri_f[:])
            identb = cst.tile([P, P], bf16, name="identb")
            nc.vector.tensor_copy(identb[:], ident_f[:])

            # channels-on-partitions activations, per (batch, T-half):
            # fine tiles let attention/outproj start before the whole
            # batch finishes (whole-tile dependency granularity)
            qa, ka, va, ya, vt = {}, {}, {}, {}, {}
            for b in range(B):
                for hf in range(2):
                    qa[b, hf] = cst.tile([P, HF], bf16, name=f"qa{b}_{hf}")
                    ka[b, hf] = cst.tile([P, HF], bf16, name=f"ka{b}_{hf}")
                    va[b, hf] = cst.tile([P, HF], bf16, name=f"va{b}_{hf}")
                    ya[b, hf] = cst.tile([P, HF], f32r, name=f"ya{b}_{hf}")
                    for h in range(2):
                        # V~ = [V | 1]: [k partitions, j, 65]
                        vt[b, h, hf] = cst.tile([P, KT // 2, D + 1], bf16,
                                                name=f"vt{b}_{h}_{hf}")
            ones8 = cst.tile([P, KT // 2], f32, name="ones8")
            nc.gpsimd.memset(ones8[:], 1.0)
            for k in list(vt):
                nc.vector.tensor_copy(vt[k][:, :, D], ones8[:])

            dest = {"q": qa, "k": ka, "v": va}

            for rep in range(repeat):
                _emit_body(nc, tc, rep, w_tiles, wo_sb, bias, identb, trimask,
                           qa, ka, va, ya, vt, dest, xT_t, out_t,
                           xtp, ptp, sbp, ps2, psy, phases,
                           prefetch if rep == 0 else None)

    nc.compile()
    return nc


def _emit_body(nc, tc, rep, w_tiles, wo_sb, bias, ident, trimask,
               qa, ka, va, ya, vt, dest, xT_t, out_t,
               xtp, ptp, sbp, ps2, psy, phases=(1, 2, 3, 4), prefetch=None):

    def proj_half(b, hf):
        # phase 1: projections for tokens [b*T + hf*HF, +HF)
        for u in (b * 4 + hf * 2, b * 4 + hf * 2 + 1):
            col0 = (u % 2) * TB
            if prefetch is not None and u in prefetch:
                xts = prefetch[u]
            else:
                xts = []
                for j in range(CT):
                    xt_ = xtp.tile([P, TB], bf16, tag=f"xt{j}",
                                   name=f"xt{j}_{u}_{rep}")
                    (nc.sync if j % 2 == 0 else nc.gpsimd).dma_start(
                        xt_[:], xT_t[j, u])
                    xts.append(xt_)
            for nm in ("q", "k", "v"):
                acc = ps2.tile([P, TB], f32, tag="proj",
                               name=f"pj_{nm}{u}_{rep}")
                for j in range(CT):
                    nc.tensor.matmul(acc[:], w_tiles[nm][j][:], xts[j][:],
                                     start=(j == 0), stop=(j == CT - 1))
                nc.vector.tensor_scalar_add(
                    dest[nm][b, hf][:, col0:col0 + TB], acc[:], bias[nm][:])

    def vt_half(b, hf):
        # phase 1.5: V transposes into [k, d] layout
        for h in range(2):
            vti = vt[b, h, hf]
            hh = h * D
            for jj in range(KT // 2):
                src = va[b, hf][hh:hh + D, jj * P:(jj + 1) * P]
                pst = ps2.tile([P, D], bf16, tag="proj",
                               name=f"tp{b}{h}{hf}{jj}_{rep}")
                nc.tensor.transpose(pst[:], src, ident[hh:hh + D, hh:hh + D])
                nc.vector.tensor_copy(vti[:, jj, 0:D], pst[:])

    def attn_chunk(b, qi):
        # phase 2: causal attention, q in [qi*QC, +QC), both local heads.
        # The two heads' score matmuls contract over row strips 0:64 and
        # 64:128 -> PE runs them concurrently; one packed exp per k-tile.
        q0 = qi * QC
        jmax = (q0 + QC) // P
        hf = qi // 2
        c0 = (qi % 2) * QC          # q offset inside the (b, hf) tiles
        y_ps = psy.tile([P, 2 * QC], f32, tag="y", name=f"y{b}{qi}_{rep}")
        for j in range(jmax):
            q_lo = max(q0, j * P)
            q_len = q0 + QC - q_lo
            co = c0 + q_lo - q0     # in-tile start col for this k-tile
            kah = ka[b, j // 8]
            kcol = (j % 8) * P
            sc = ps2.tile([P, 2 * QC], f32, tag="sc",
                          name=f"sc{b}{qi}_{j}_{rep}")
            for h in range(2):
                hh = h * D
                nc.tensor.matmul(
                    sc[:, h * QC:h * QC + q_len],
                    kah[hh:hh + D, kcol:kcol + P],
                    qa[b, hf][hh:hh + D, co:co + q_len],
                    start=True, stop=True)
            pt = ptp.tile([P, 2, QC], bf16, tag="pt",
                          name=f"pt{b}{qi}_{j}_{rep}")
            nc.scalar.activation(
                pt[:, :, 0:q_len],
                sc[:].rearrange("p (h q) -> p h q", h=2)[:, :, 0:q_len],
                EXP, scale=SCALE)
            if j * P >= q0:
                # diagonal block: zero strictly-upper (q < k)
                for h in range(2):
                    nc.vector.tensor_mul(pt[:, h, 0:P], pt[:, h, 0:P],
                                         trimask[:])
            q_abs_end = q_lo + q_len
            j_last = (q_abs_end - 1) // P
            for h in range(2):
                nc.tensor.matmul(
                    y_ps[0:D + 1, h * QC + q_lo - q0:h * QC + QC],
                    vt[b, h, j // 8][:, j % 8, :],
                    pt[:, h, 0:q_len],
                    start=(j == 0), stop=(j == j_last))
        # softmax divide: row D of y_ps holds both heads' denominators
        rc = sbp.tile([1, 2 * QC], f32, tag="rc", name=f"rc{b}{qi}_{rep}")
        nc.vector.reciprocal(rc[:], y_ps[D:D + 1, :])
        bc = sbp.tile([D, 2 * QC], f32, tag="bc", name=f"bc{b}{qi}_{rep}")
        nc.gpsimd.partition_broadcast(bc[:], rc[:], channels=D)
        for h in range(2):
            nc.vector.tensor_mul(ya[b, hf][h * D:h * D + D, c0:c0 + QC],
                                 y_ps[0:D, h * QC:(h + 1) * QC],
                                 bc[:, h * QC:(h + 1) * QC])

    def out_half(b, hf):
        # phase 3: partial out-projection for 8 row-tiles of this half
        for s in range(hf * 8, hf * 8 + 8):
            o_sb = sbp.tile([P, C], f32, tag="osb", name=f"o{b}{s}_{rep}")
            ya_t = ya[b, hf][:, (s % 8) * P:(s % 8 + 1) * P]
            for m in range(2):
                acc = ps2.tile([P, 512], f32, tag="proj",
                               name=f"op{b}{s}{m}_{rep}")
                nc.tensor.matmul(acc[:], ya_t, wo_sb[:, m * 512:(m + 1) * 512],
                                 start=True, stop=True)
                nc.any.tensor_copy(o_sb[:, m * 512:(m + 1) * 512], acc[:])
            (nc.sync if s % 2 == 0 else nc.gpsimd).dma_start(
                out_t[b * (T // P) + s], o_sb[:])

    # Emission order = scheduler priority. Interleave so the attention
    # chunks always have PE/DMA work (projections, transposes,
    # out-projection) available to fill gaps, and each stage starts as
    # soon as its half-granular inputs are ready.
    P1 = 1 in phases
    P15 = 2 in phases
    P2 = 3 in phases
    P3 = 4 in phases
    if P1:
        proj_half(0, 0)
    if P15:
        vt_half(0, 0)
    if P2:
        attn_chunk(0, 0)
        attn_chunk(0, 1)
    if P1:
        proj_half(0, 1)
    if P15:
        vt_half(0, 1)
    if P2:
        attn_chunk(0, 2)
    if P1:
        proj_half(1, 0)
    if P15:
        vt_half(1, 0)
    if P2:
        attn_chunk(0, 3)
    if P1:
        proj_half(1, 1)
    if P15:
        vt_half(1, 1)
    if P2:
        attn_chunk(1, 0)
        attn_chunk(1, 1)
    if P3:
        out_half(0, 0)
    if P2:
        attn_chunk(1, 2)
    if P3:
        out_half(0, 1)
    if P2:
        attn_chunk(1, 3)
    if P3:
        out_half(1, 0)
        out_half(1, 1)


def get_nc():
    global _CACHED_NC
    if _CACHED_NC is None:
        _CACHED_NC = _build_nc()
    return _CACHED_NC


def make_in_maps(x, Wq, bq, Wk, bk, Wv, bv, Wo, bo):
    x2 = np.ascontiguousarray(np.asarray(x, np.float32).reshape(NTOK, C).T)
    x2b = x2.astype(ml_dtypes.bfloat16)
    Wq = np.asarray(Wq, np.float32)
    Wk = np.asarray(Wk, np.float32)
    Wv = np.asarray(Wv, np.float32)
    Wo = np.asarray(Wo, np.float32)
    in_maps = []
    for i in range(NCORES):
        ci = slice(i * P, (i + 1) * P)
        in_maps.append({
            "xT": x2b,
            "wqT": np.ascontiguousarray(Wq[ci].T).astype(ml_dtypes.bfloat16),
            "wkT": np.ascontiguousarray(Wk[ci].T).astype(ml_dtypes.bfloat16),
            "wvT": np.ascontiguousarray(Wv[ci].T).astype(ml_dtypes.bfloat16),
            "woT": np.ascontiguousarray(Wo[:, ci].T),
            "bq": np.asarray(bq, np.float32)[ci].reshape(P, 1),
            "bk": np.asarray(bk, np.float32)[ci].reshape(P, 1),
            "bv": np.asarray(bv, np.float32)[ci].reshape(P, 1),
        })
    return in_maps


def kernel(x, Wq, bq, Wk, bk, Wv, bv, Wo, bo):
    nc = get_nc()
    in_maps = make_in_maps(x, Wq, bq, Wk, bk, Wv, bv, Wo, bo)
    res = run_bass_kernel_spmd(nc, in_maps, list(range(NCORES)))
    acc = np.zeros((NTOK, C), np.float64)
    for i in range(NCORES):
        acc += res.results[i]["out"].astype(np.float64)
    acc += np.asarray(bo, np.float64)[None, :]
    return acc.reshape(B, T, C).astype(np.float32)
